# revision 21
# baseline (speedup 1.0000x reference)
"""AttentionDTI forward pass on 8 TRN2 NeuronCores — pure data parallel over batch.

Model (B=8, LD=100, LP=1000, DIM=64, CONV=40, C4=160):
  embed -> 3x conv1d+relu (drug: k=4,6,8 ; protein: k=4,8,12)
  d_att = dc^T @ d_att_w + b ; p_att = pc^T @ p_att_w + b
  R = relu(d_att[:,i,None,:] + p_att[:,None,j,:])      # [B,85,979,160] never materialized
  comp_atte = sigmoid((R.mean(2) @ att_w + att_b)^T)   # via S[c,i] = sum_j relu(...)
  prot_atte = sigmoid((R.mean(1) @ att_w + att_b)^T)   # via T[c,j] = sum_i relu(...)
  gate, global max pool, FC 320->1024->1024->512->2 (leaky relu 0.01)

Sharding: core b handles batch element b. All params replicated. No collectives.

v2 changes vs v1 (190.8us -> 137.7us):
  - All parameters are packed host-side into 5 large [128, W] DRAM tensors +
    one idx row and loaded with 6 DMAs (was ~105 small DMAs at ~650ns issue
    cost each on the sync HWDGE ring, which stalled all compute for ~45us).
    idx goes on the scalar HWDGE ring, packs on the sync ring in order of use.
  - No PE warm-up: the HW limits the PE to a 50% utilization duty cycle for
    the first ~50us of accumulated PE activity regardless, so warm-up matmuls
    only burn that budget and delay real work queued behind them.
  - conv3 and the attention projections are fused into one L-chunked loop so
    the projection matmuls/activations overlap the next conv3 chunk.
  - R-loop producers alternate DVE scalar_tensor_tensor / ScalarE activation
    (the only ops that fuse relu with a sum-accumulate; both ~1.1-1.2us per
    [128,980] pass, both engines ~100% busy -> producer-bound floor).
    Protein att tiles are padded to 980 cols with -1e4 so relu(pad) = 0.
  - Gate (0.5 + atte) * src fused into one scalar_tensor_tensor; Sigmoid act
    table warmed at boot; T PSUM->SBUF copies split across ScalarE/DVE.
"""

import numpy as np

B, LD, LP, DIM, CONV = 8, 100, 1000, 64, 40
C4 = 160
LD1, LD2, LD3 = 97, 92, 85     # drug conv output lengths (k=4,6,8)
LP1, LP2, LP3 = 997, 990, 979  # protein conv output lengths (k=4,8,12)
LPP = 980                      # padded (even) protein length for the R loop
NB = 22                        # ceil(85/4) packed iterations for chunk B
# R-iter producer schedule, repeating: V=DVE (scalar_tensor_tensor),
# A=ScalarE (activation). Both are ~1.1-1.2us per [128,980] pass — the only
# ops that fuse relu with a sum-accumulate; neuronxcc rejects them on Pool.
R_SCHED = "VA"

CH = [(0, 128), (128, 32)]     # (offset, width) chunks of the 160 dim

_CACHE = {}


def _mk_pack(entries):
    """entries: [(name, rows, cols)] -> ({name: (rows, off, cols)}, width)."""
    d, off = {}, 0
    for name, r, c in entries:
        d[name] = (r, off, c)
        off += c
    return d, off


PK_F32 = _mk_pack(
    [("iota", 128, 1),
     ("db1", CONV, 1), ("db2", 2 * CONV, 1), ("db3A", 128, 1), ("db3B", 32, 1),
     ("pb1", CONV, 1), ("pb2", 2 * CONV, 1), ("pb3A", 128, 1), ("pb3B", 32, 1),
     ("dabA", 128, 1), ("dabB", 32, 1), ("pabA", 128, 1), ("pabB", 32, 1),
     ("abA", 128, 1), ("abB", 32, 1), ("dabr", 128, 1), ("pabr", 128, 1),
     ("fc1b", 128, 8), ("fc2b", 128, 8), ("fc3b", 128, 4), ("outb", 2, 1)])

PK_BOOT = _mk_pack(
    [("ones", 1, 128), ("embd", 65, DIM), ("embp", 26, DIM),
     ("id128", 128, 128), ("id4", 128, 32)])

PK_CONV = _mk_pack(
    [(f"dw1_{k}", DIM, CONV) for k in range(4)]
    + [(f"dw2_{k}", CONV, 2 * CONV) for k in range(6)]
    + [(f"dw3_{k}", 2 * CONV, C4) for k in range(8)]
    + [(f"pw1_{k}", DIM, CONV) for k in range(4)]
    + [(f"pw2_{k}", CONV, 2 * CONV) for k in range(8)]
    + [(f"pw3_{k}", 2 * CONV, C4) for k in range(12)])

PK_ATT = _mk_pack(
    [("dawA", 128, C4), ("dawB", 32, C4), ("pawA", 128, C4), ("pawB", 32, C4),
     ("awA", 128, C4), ("awB", 32, C4),
     ("dawrA", 128, 128), ("dawrB", 32, 128),
     ("pawrA", 128, 128), ("pawrB", 32, 128)])

PK_FC = _mk_pack(
    [("fc1_0", 128, 1024), ("fc1_1", 32, 1024),
     ("fc1_2", 128, 1024), ("fc1_3", 32, 1024)]
    + [(f"fc2_{g}", 128, 1024) for g in range(8)]
    + [(f"fc3_{g}", 128, 512) for g in range(8)]
    + [(f"outw_{g}", 128, 2) for g in range(4)])


def _build():
    from contextlib import ExitStack
    import concourse.bass as bass
    import concourse.tile as tile
    from concourse import bacc, mybir

    f32 = mybir.dt.float32
    bf16 = mybir.dt.bfloat16
    AF = mybir.ActivationFunctionType
    ALU = mybir.AluOpType
    AX = mybir.AxisListType

    nc = bacc.Bacc("TRN2", target_bir_lowering=False, debug=False)

    idx_d = nc.declare_dram_parameter("idx", [1, 1104], bf16, isOutput=False)
    pk_d = {}
    for pname, (layout, w), dt in [
        ("pk_f32", PK_F32, f32), ("pk_boot", PK_BOOT, bf16),
        ("pk_conv", PK_CONV, bf16), ("pk_att", PK_ATT, bf16),
        ("pk_fc", PK_FC, bf16),
    ]:
        pk_d[pname] = nc.declare_dram_parameter(pname, [128, w], dt, isOutput=False)
    out_d = nc.declare_dram_parameter("out", [2, 1], f32, isOutput=True)

    with tile.TileContext(nc) as tc, ExitStack() as ctx:
        wp = ctx.enter_context(tc.tile_pool(name="w", bufs=1))
        ap_ = ctx.enter_context(tc.tile_pool(name="a", bufs=1))
        tp = ctx.enter_context(tc.tile_pool(name="t", bufs=8))
        pp = ctx.enter_context(tc.tile_pool(name="p", bufs=4, space="PSUM"))
        pT = ctx.enter_context(tc.tile_pool(name="pT", bufs=1, space="PSUM"))

        # ---- coalesced loads: idx on the scalar HWDGE ring (runs in
        # parallel with the sync ring's packs), packs in order of use ----
        idx_t = ap_.tile([1, 1104], bf16, tag="idx")
        nc.scalar.dma_start(out=idx_t[:], in_=idx_d[:])
        pk_t = {}
        for pname, (layout, w), dt in [
            ("pk_boot", PK_BOOT, bf16), ("pk_f32", PK_F32, f32),
            ("pk_conv", PK_CONV, bf16), ("pk_att", PK_ATT, bf16),
            ("pk_fc", PK_FC, bf16),
        ]:
            t = wp.tile([128, w], dt, tag=pname)
            nc.sync.dma_start(out=t[:], in_=pk_d[pname][:])
            pk_t[pname] = t

        def sl(pname, name):
            layout, _ = {"pk_f32": PK_F32, "pk_boot": PK_BOOT,
                         "pk_conv": PK_CONV, "pk_att": PK_ATT,
                         "pk_fc": PK_FC}[pname]
            r, off, c = layout[name]
            return pk_t[pname][0:r, off:off + c]

        # No PE warm-up: the HW runs the PE at a 50% utilization duty cycle
        # for the first ~50us of accumulated PE activity regardless (throttle
        # telemetry shows util_limit=0.5), so warm-up matmuls only burn that
        # budget and delay the real work behind them in the PE queue.
        ones_t = sl("pk_boot", "ones")
        iota_t = sl("pk_f32", "iota")
        id_t = sl("pk_boot", "id128")
        id4_t = sl("pk_boot", "id4")

        # ---- one-hot + embedding ----
        def embed(idx_ap, nvocab, L, emb_ap, tag):
            e = ap_.tile([DIM, L], bf16, tag=f"e_{tag}")
            for l0 in range(0, L, 512):
                cs = min(512, L - l0)
                psb = pp.tile([nvocab, 512], f32, tag="ps")
                nc.tensor.matmul(psb[:, :cs], ones_t[:, :nvocab],
                                 idx_ap[:, l0:l0 + cs], start=True, stop=True)
                oh = tp.tile([nvocab, 512], bf16, tag="oh")
                nc.vector.tensor_scalar(out=oh[:, :cs], in0=psb[:, :cs],
                                        scalar1=iota_t[:nvocab, :], scalar2=None,
                                        op0=ALU.is_equal)
                pse = pp.tile([DIM, 512], f32, tag="ps")
                nc.tensor.matmul(pse[:, :cs], emb_ap, oh[:, :cs], start=True, stop=True)
                nc.scalar.copy(e[:, l0:l0 + cs], pse[:, :cs])
            return e

        de = embed(idx_t[:, 0:LD], 65, LD, sl("pk_boot", "embd"), "d")
        pe = embed(idx_t[:, LD:LD + LP], 26, LP, sl("pk_boot", "embp"), "p")

        # ---- conv stacks (bf16 in/out, f32 psum) ----
        def conv(x, Lout, K, wname, b_ap, cout, tag, oc=None):
            y = ap_.tile([cout, Lout], bf16, tag=tag)
            for l0 in range(0, Lout, 512):
                cs = min(512, Lout - l0)
                ps = pp.tile([cout, 512], f32, tag="ps")
                for k in range(K):
                    w = sl("pk_conv", f"{wname}_{k}")
                    if oc is not None:
                        w = w[:, oc[0]:oc[0] + oc[1]]
                    nc.tensor.matmul(ps[:, :cs], w, x[:, l0 + k:l0 + k + cs],
                                     start=(k == 0), stop=(k == K - 1))
                nc.scalar.activation(y[:, l0:l0 + cs], ps[:, :cs], AF.Relu, bias=b_ap)
            return y

        dc1 = conv(de, LD1, 4, "dw1", sl("pk_f32", "db1"), CONV, "dc1")
        dc2 = conv(dc1, LD2, 6, "dw2", sl("pk_f32", "db2"), 2 * CONV, "dc2")
        pc1 = conv(pe, LP1, 4, "pw1", sl("pk_f32", "pb1"), CONV, "pc1")
        pc2 = conv(pc1, LP2, 8, "pw2", sl("pk_f32", "pb2"), 2 * CONV, "pc2")

        # ---- fused conv3 + attention projections, chunked along L so the
        # projection matmuls/activations overlap the next conv3 chunk ----
        # out tiles: X_A [128, L] (chans 0:128) and X_B4 [128, L] (chans
        # 128:160 x4 lane-replicated). Protein att tiles are [128, LPP] with
        # col 979 = -1e4 so relu() of the pad contributes 0.
        def conv3_att(x, L, Lpad, K, wname, pfx, tag, dt_a):
            cc0 = ap_.tile([CH[0][1], L], bf16, tag=f"{tag}c0")
            cc1 = ap_.tile([CH[1][1], L], bf16, tag=f"{tag}c1")
            cc = [cc0, cc1]
            aA = ap_.tile([128, Lpad], dt_a, tag=f"{tag}a0")
            aB = ap_.tile([128, Lpad], dt_a, tag=f"{tag}a1")
            if Lpad > L:
                nc.vector.memset(aA[:, L:Lpad], -1e4)
                nc.vector.memset(aB[:, L:Lpad], -1e4)
            for l0 in range(0, L, 512):
                cs = min(512, L - l0)
                for j, s in ((0, "A"), (1, "B")):
                    o, w_ = CH[j]
                    ps = pp.tile([w_, 512], f32, tag="ps")
                    for k in range(K):
                        w = sl("pk_conv", f"{wname}_{k}")[:, o:o + w_]
                        nc.tensor.matmul(ps[:, :cs], w, x[:, l0 + k:l0 + k + cs],
                                         start=(k == 0), stop=(k == K - 1))
                    nc.scalar.activation(cc[j][:, l0:l0 + cs], ps[:, :cs], AF.Relu,
                                         bias=sl("pk_f32", f"{pfx}b3{s}"))
                for which, y in ((0, aA), (1, aB)):
                    ps = pp.tile([128, 512], f32, tag="ps")
                    for j, s in ((0, "A"), (1, "B")):
                        w = (sl("pk_att", f"{pfx}awA")[:, 0:128],
                             sl("pk_att", f"{pfx}awB")[:, 0:128])[j] if which == 0 \
                            else (sl("pk_att", f"{pfx}awrA"),
                                  sl("pk_att", f"{pfx}awrB"))[j]
                        nc.tensor.matmul(ps[:, :cs], w, cc[j][:, l0:l0 + cs],
                                         start=(j == 0), stop=(j == 1))
                    bias = sl("pk_f32", f"{pfx}abA") if which == 0 \
                        else sl("pk_f32", f"{pfx}abr")
                    nc.scalar.activation(y[:, l0:l0 + cs], ps[:, :cs], AF.Identity,
                                         bias=bias)
            return cc, aA, aB

        # D tiles f32 (used as per-partition scalars); P tiles bf16 (streamed)
        dc, D_A, D_B4 = conv3_att(dc2, LD3, LD3, 8, "dw3", "d", "dc3", f32)
        pc, P_A, P_B4 = conv3_att(pc2, LP3, LPP, 12, "pw3", "p", "pc3", bf16)

        # pack D_B4 [128, 85] -> D_Bp [128, 22]: lane (32g+c), col t = D[128+c, 4t+g]
        D_Bpad = ap_.tile([128, 88], f32, tag="D_Bpad")
        nc.vector.memset(D_Bpad[:], -1e4)
        nc.vector.tensor_copy(D_Bpad[:, 0:85], D_B4[:, 0:85])
        D_Bp = ap_.tile([128, NB], f32, tag="D_Bp")
        for g in range(4):
            nc.vector.tensor_copy(D_Bp[g * 32:(g + 1) * 32, :],
                                  D_Bpad[g * 32:(g + 1) * 32, g:88:4])

        # ---- R loops ----
        # tmp = relu(P + D[:, i]); S col via in-instruction accumulate;
        # T += tmp via identity matmul into PSUM. Producers alternate between
        # DVE scalar_tensor_tensor and ScalarE activation (both ~1.1-1.2us for
        # a [128,980] pass; no DVE fast mode exists for any op that can fuse
        # relu with a sum-accumulate).
        zeros_t = ap_.tile([128, LPP], bf16, tag="zeros")
        nc.vector.memset(zeros_t[:], 0.0)

        # Warm the Sigmoid activation table off the critical path (its
        # ACT_TABLE_LOAD is ~1.3us and would otherwise fire at first atte use)
        sig_wu = ap_.tile([1, 2], f32, tag="sig_wu")
        nc.scalar.activation(sig_wu[:], zeros_t[0:1, 0:2], AF.Sigmoid)

        def r_loop(P_t, D_cols, n_iter, s_tile, psl, psh, id_tile, idw):
            for i in range(n_iter):
                tm = tp.tile([128, LPP], bf16, tag="rtmp")
                eng = R_SCHED[i % len(R_SCHED)]
                if eng == "A":
                    nc.scalar.activation(tm[:], P_t[:], AF.Relu,
                                         bias=D_cols[:, i:i + 1],
                                         accum_out=s_tile[:, i:i + 1])
                else:
                    nc.vector.scalar_tensor_tensor(
                        out=tm[:], in0=P_t[:], scalar=D_cols[:, i:i + 1],
                        in1=zeros_t[:], op0=ALU.add, op1=ALU.max,
                        accum_out=s_tile[:, i:i + 1])
                nc.tensor.matmul(psl[:], id_tile[:, :idw], tm[:, 0:512],
                                 start=(i == 0), stop=(i == n_iter - 1))
                nc.tensor.matmul(psh[:], id_tile[:, :idw], tm[:, 512:LPP],
                                 start=(i == 0), stop=(i == n_iter - 1))

        S_A = ap_.tile([128, LD3], f32, tag="S_A")
        TA0 = pT.tile([128, 512], f32, tag="TA0")
        TA1 = pT.tile([128, LPP - 512], f32, tag="TA1")
        r_loop(P_A, D_A, LD3, S_A, TA0, TA1, id_t, 128)

        S_B4 = ap_.tile([128, NB], f32, tag="S_B4")
        TB0 = pT.tile([32, 512], f32, tag="TB0")
        TB1 = pT.tile([32, LPP - 512], f32, tag="TB1")
        r_loop(P_B4, D_Bp, NB, S_B4, TB0, TB1, id4_t, 32)

        # S -> bf16 rhs tiles: S_Ab [128, 85]; unpack S_B4 -> S_Bb [32, 85]
        S_Ab = ap_.tile([128, LD3], bf16, tag="S_Ab")
        nc.vector.tensor_copy(S_Ab[:], S_A[:])
        S_Bb = ap_.tile([32, LD3], bf16, tag="S_Bb")
        for g in range(4):
            cnt = NB if g == 0 else NB - 1
            nc.vector.tensor_copy(S_Bb[:, g:g + 4 * (cnt - 1) + 1:4],
                                  S_B4[g * 32:(g + 1) * 32, 0:cnt])
        # T psum -> bf16 sbuf (pad col 979 dropped); A on ScalarE, B on DVE so
        # the two copies overlap
        T_Ab = ap_.tile([128, LP3], bf16, tag="T_Ab")
        nc.scalar.copy(T_Ab[:, 0:512], TA0[:])
        nc.scalar.copy(T_Ab[:, 512:LP3], TA1[:, 0:LP3 - 512])
        T_Bb = ap_.tile([32, LP3], bf16, tag="T_Bb")
        nc.vector.tensor_copy(T_Bb[:, 0:512], TB0[:])
        nc.vector.tensor_copy(T_Bb[:, 512:LP3], TB1[:, 0:LP3 - 512])
        S_ch = [S_Ab, S_Bb]
        T_ch = [T_Ab, T_Bb]

        # ---- attention outputs: sigmoid((sum/n) @ att_w + att_b) ----
        def atte(rhs_ch, L, scale, tag):
            res = []
            for which, (o, w) in enumerate(CH):
                y = ap_.tile([w, L], bf16, tag=f"{tag}{which}")
                for l0 in range(0, L, 512):
                    cs = min(512, L - l0)
                    ps = pp.tile([w, 512], f32, tag="ps")
                    for j, s in ((0, "A"), (1, "B")):
                        aw = sl("pk_att", f"aw{s}")
                        nc.tensor.matmul(ps[:, :cs], aw[:, o:o + w],
                                         rhs_ch[j][:, l0:l0 + cs],
                                         start=(j == 0), stop=(j == 1))
                    nc.scalar.activation(y[:, l0:l0 + cs], ps[:, :cs], AF.Sigmoid,
                                         bias=sl("pk_f32", f"ab{'AB'[which]}"),
                                         scale=scale)
                res.append(y)
            return res

        ca = atte(S_ch, LD3, 1.0 / LP3, "ca")
        pa = atte(T_ch, LP3, 1.0 / LD3, "pa")

        # ---- gate + global max pool: v = max_l(src * (0.5 + atte)) ----
        vecs = {}
        for (src, att_, L, tag) in [(dc, ca, LD3, "d"), (pc, pa, LP3, "p")]:
            for which, (o, w) in enumerate(CH):
                m = tp.tile([w, L], bf16, tag=f"m_{tag}{which}")
                nc.vector.scalar_tensor_tensor(
                    out=m[:], in0=att_[which][:], scalar=0.5,
                    in1=src[which][:, 0:L], op0=ALU.add, op1=ALU.mult)
                v = ap_.tile([w, 1], bf16, tag=f"v_{tag}{which}")
                nc.vector.reduce_max(v[:], m[:], axis=AX.X)
                vecs[f"{tag}{which}"] = v
        # pair layout: [dvecA(128), dvecB(32), pvecA(128), pvecB(32)]
        vlist = [vecs["d0"], vecs["d1"], vecs["p0"], vecs["p1"]]

        # ---- FC head ----
        def lrelu_bias(ps, b_ap, ncols, tag):
            h = ap_.tile([128, ncols], f32, tag=f"h_{tag}")
            nc.vector.tensor_tensor(out=h[:], in0=ps[:, :ncols], in1=b_ap, op=ALU.add)
            t1 = tp.tile([128, ncols], f32, tag="fct")
            nc.vector.tensor_scalar(out=t1[:], in0=h[:], scalar1=0.01, scalar2=None,
                                    op0=ALU.mult)
            h2 = ap_.tile([128, ncols], bf16, tag=f"h2_{tag}")
            nc.vector.tensor_tensor(out=h2[:], in0=h[:], in1=t1[:], op=ALU.max)
            return h2

        ps1 = pp.tile([128, 8], f32, tag="ps")
        for oc in range(8):
            for g in range(4):
                w = sl("pk_fc", f"fc1_{g}")
                nc.tensor.matmul(ps1[:, oc:oc + 1], w[:, oc * 128:(oc + 1) * 128],
                                 vlist[g][:], start=(g == 0), stop=(g == 3))
        h1 = lrelu_bias(ps1, sl("pk_f32", "fc1b"), 8, "1")

        ps2 = pp.tile([128, 8], f32, tag="ps")
        for oc in range(8):
            for g in range(8):
                w = sl("pk_fc", f"fc2_{g}")
                nc.tensor.matmul(ps2[:, oc:oc + 1], w[:, oc * 128:(oc + 1) * 128],
                                 h1[:, g:g + 1], start=(g == 0), stop=(g == 7))
        h2 = lrelu_bias(ps2, sl("pk_f32", "fc2b"), 8, "2")

        ps3 = pp.tile([128, 4], f32, tag="ps")
        for oc in range(4):
            for g in range(8):
                w = sl("pk_fc", f"fc3_{g}")
                nc.tensor.matmul(ps3[:, oc:oc + 1], w[:, oc * 128:(oc + 1) * 128],
                                 h2[:, g:g + 1], start=(g == 0), stop=(g == 7))
        h3 = lrelu_bias(ps3, sl("pk_f32", "fc3b"), 4, "3")

        pso = pp.tile([2, 1], f32, tag="ps")
        for g in range(4):
            nc.tensor.matmul(pso[:], sl("pk_fc", f"outw_{g}"), h3[:, g:g + 1],
                             start=(g == 0), stop=(g == 3))
        ob = ap_.tile([2, 1], f32, tag="ob")
        nc.scalar.activation(ob[:], pso[:], AF.Identity, bias=sl("pk_f32", "outb"))
        nc.sync.dma_start(out=out_d[:], in_=ob[:])

    nc.compile()
    return nc


def _prep_inputs(inputs):
    """Host-side layout prep. Returns (shared_params, per_core_fn)."""
    import ml_dtypes
    bf = ml_dtypes.bfloat16
    asn = np.asarray
    rep4 = lambda x: np.tile(x, (4,) + (1,) * (x.ndim - 1))

    vals = {}
    # f32 pack values
    vals["iota"] = np.arange(128, dtype=np.float32).reshape(128, 1)
    for nm, src in [("db1", "db1"), ("db2", "db2"), ("pb1", "pb1"), ("pb2", "pb2")]:
        vals[nm] = asn(inputs[src], dtype=np.float32).reshape(-1, 1)
    for nm, src in [("db3", "db3"), ("pb3", "pb3"), ("dab", "d_att_b"),
                    ("pab", "p_att_b"), ("ab", "att_b")]:
        v = asn(inputs[src], dtype=np.float32).reshape(-1, 1)
        vals[nm + "A"], vals[nm + "B"] = v[0:128], v[128:160]
    vals["dabr"] = rep4(asn(inputs["d_att_b"], dtype=np.float32)[128:160]).reshape(128, 1)
    vals["pabr"] = rep4(asn(inputs["p_att_b"], dtype=np.float32)[128:160]).reshape(128, 1)
    vals["fc1b"] = asn(inputs["fc1_b"], dtype=np.float32).reshape(8, 128).T.copy()
    vals["fc2b"] = asn(inputs["fc2_b"], dtype=np.float32).reshape(8, 128).T.copy()
    vals["fc3b"] = asn(inputs["fc3_b"], dtype=np.float32).reshape(4, 128).T.copy()
    vals["outb"] = asn(inputs["out_b"], dtype=np.float32).reshape(2, 1)
    # boot pack
    vals["ones"] = np.ones((1, 128), np.float32)
    vals["embd"] = asn(inputs["drug_emb"])
    vals["embp"] = asn(inputs["prot_emb"])
    vals["id128"] = np.eye(128, dtype=np.float32)
    vals["id4"] = np.tile(np.eye(32, dtype=np.float32), (4, 1))
    # conv pack: tap k of w [Cout, Cin, K] -> [Cin, Cout]
    for nm, src, K in [("dw1", "dw1", 4), ("dw2", "dw2", 6), ("dw3", "dw3", 8),
                       ("pw1", "pw1", 4), ("pw2", "pw2", 8), ("pw3", "pw3", 12)]:
        w = asn(inputs[src])
        for k in range(K):
            vals[f"{nm}_{k}"] = w[:, :, k].T
    # att pack
    for nm, src in [("daw", "d_att_w"), ("paw", "p_att_w"), ("aw", "att_w")]:
        w = asn(inputs[src])
        vals[nm + "A"], vals[nm + "B"] = w[0:128], w[128:160]
    for nm, src in [("dawr", "d_att_w"), ("pawr", "p_att_w")]:
        w = np.tile(asn(inputs[src])[:, 128:160], (1, 4))
        vals[nm + "A"], vals[nm + "B"] = w[0:128], w[128:160]
    # fc pack
    fc1 = asn(inputs["fc1_w"])
    vals["fc1_0"], vals["fc1_1"] = fc1[0:128], fc1[128:160]
    vals["fc1_2"], vals["fc1_3"] = fc1[160:288], fc1[288:320]
    fc2, fc3 = asn(inputs["fc2_w"]), asn(inputs["fc3_w"])
    for g in range(8):
        vals[f"fc2_{g}"] = fc2[g * 128:(g + 1) * 128]
        vals[f"fc3_{g}"] = fc3[g * 128:(g + 1) * 128]
    outw = asn(inputs["out_w"])
    for g in range(4):
        vals[f"outw_{g}"] = outw[g * 128:(g + 1) * 128]

    shared = {}
    for pname, (layout, w), dt in [
        ("pk_f32", PK_F32, np.float32), ("pk_boot", PK_BOOT, bf),
        ("pk_conv", PK_CONV, bf), ("pk_att", PK_ATT, bf), ("pk_fc", PK_FC, bf),
    ]:
        buf = np.zeros((128, w), dtype=dt)
        for name, (r, off, c) in layout.items():
            buf[0:r, off:off + c] = vals[name]
        shared[pname] = buf

    drug = asn(inputs["drug"]).astype(bf)
    prot = asn(inputs["protein"]).astype(bf)

    def per_core(i):
        m = dict(shared)
        idx = np.zeros((1, 1104), dtype=bf)
        idx[0, 0:LD] = drug[i]
        idx[0, LD:LD + LP] = prot[i]
        m["idx"] = idx
        return m

    return shared, per_core


def kernel(**inputs):
    import os
    # A NeuronCore left in a degraded DVFS state by a previous crash runs
    # ~20% slower; request a core reset on runtime init (no-op if the
    # harness already set a policy).
    os.environ.setdefault("NEURON_RT_RESET_CORES", "1")
    from concourse.bass_utils import run_bass_kernel_spmd

    if "nc" not in _CACHE:
        _CACHE["nc"] = _build()
    nc = _CACHE["nc"]
    _, per_core = _prep_inputs(inputs)
    in_maps = [per_core(i) for i in range(B)]
    r = run_bass_kernel_spmd(nc, in_maps, core_ids=list(range(B)))
    out = np.stack([r.results[i]["out"].reshape(2) for i in range(B)])
    return out.astype(np.float32)


# revision 26
# speedup vs baseline: 1.0194x; 1.0194x over previous
"""AttentionDTI forward pass on 8 TRN2 NeuronCores — pure data parallel over batch.

Model (B=8, LD=100, LP=1000, DIM=64, CONV=40, C4=160):
  embed -> 3x conv1d+relu (drug: k=4,6,8 ; protein: k=4,8,12)
  d_att = dc^T @ d_att_w + b ; p_att = pc^T @ p_att_w + b
  R = relu(d_att[:,i,None,:] + p_att[:,None,j,:])      # [B,85,979,160] never materialized
  comp_atte = sigmoid((R.mean(2) @ att_w + att_b)^T)   # via S[c,i] = sum_j relu(...)
  prot_atte = sigmoid((R.mean(1) @ att_w + att_b)^T)   # via T[c,j] = sum_i relu(...)
  gate, global max pool, FC 320->1024->1024->512->2 (leaky relu 0.01)

Sharding: core b handles batch element b. All params replicated. No collectives.

v2 changes vs v1 (190.8us -> 137.7us):
  - All parameters are packed host-side into 5 large [128, W] DRAM tensors +
    one idx row and loaded with 6 DMAs (was ~105 small DMAs at ~650ns issue
    cost each on the sync HWDGE ring, which stalled all compute for ~45us).
    idx goes on the scalar HWDGE ring, packs on the sync ring in order of use.
  - No PE warm-up: the HW limits the PE to a 50% utilization duty cycle for
    the first ~50us of accumulated PE activity regardless, so warm-up matmuls
    only burn that budget and delay real work queued behind them.
  - conv3 and the attention projections are fused into one L-chunked loop so
    the projection matmuls/activations overlap the next conv3 chunk.
  - R-loop producers alternate DVE scalar_tensor_tensor / ScalarE activation
    (the only ops that fuse relu with a sum-accumulate; both ~1.1-1.2us per
    [128,980] pass, both engines ~100% busy -> producer-bound floor).
    Protein att tiles are padded to 980 cols with -1e4 so relu(pad) = 0.
  - Gate (0.5 + atte) * src fused into one scalar_tensor_tensor; Sigmoid act
    table warmed at boot; T PSUM->SBUF copies split across ScalarE/DVE.
"""

import numpy as np

B, LD, LP, DIM, CONV = 8, 100, 1000, 64, 40
C4 = 160
LD1, LD2, LD3 = 97, 92, 85     # drug conv output lengths (k=4,6,8)
LP1, LP2, LP3 = 997, 990, 979  # protein conv output lengths (k=4,8,12)
LPP = 980                      # padded (even) protein length for the R loop
NB = 22                        # ceil(85/4) packed iterations for chunk B
# R-iter producer schedule, repeating: V=DVE (scalar_tensor_tensor),
# A=ScalarE (activation). Both are ~1.1-1.2us per [128,980] pass — the only
# ops that fuse relu with a sum-accumulate; neuronxcc rejects them on Pool.
R_SCHED = "VA"

CH = [(0, 128), (128, 32)]     # (offset, width) chunks of the 160 dim

_CACHE = {}


def _mk_pack(entries):
    """entries: [(name, rows, cols)] -> ({name: (rows, off, cols)}, width)."""
    d, off = {}, 0
    for name, r, c in entries:
        d[name] = (r, off, c)
        off += c
    return d, off


PK_F32 = _mk_pack(
    [("iota", 128, 1),
     ("db1", CONV, 1), ("db2", 2 * CONV, 1), ("db3A", 128, 1), ("db3B", 32, 1),
     ("pb1", CONV, 1), ("pb2", 2 * CONV, 1), ("pb3A", 128, 1), ("pb3B", 32, 1),
     ("dabA", 128, 1), ("dabB", 32, 1), ("pabA", 128, 1), ("pabB", 32, 1),
     ("abA", 128, 1), ("abB", 32, 1), ("dabr", 128, 1), ("pabr", 128, 1),
     ("fc1b", 128, 8), ("fc2b", 128, 8), ("fc3b", 128, 4), ("outb", 2, 1)])

PK_BOOT = _mk_pack(
    [("ones", 1, 128), ("embd", 65, DIM), ("embp", 26, DIM),
     ("id128", 128, 128), ("id4", 128, 32)])

PK_CONV = _mk_pack(
    [(f"dw1_{k}", DIM, CONV) for k in range(4)]
    + [(f"dw2_{k}", CONV, 2 * CONV) for k in range(6)]
    + [(f"dw3_{k}", 2 * CONV, C4) for k in range(8)]
    + [(f"pw1_{k}", DIM, CONV) for k in range(4)]
    + [(f"pw2_{k}", CONV, 2 * CONV) for k in range(8)]
    + [(f"pw3_{k}", 2 * CONV, C4) for k in range(12)])

PK_ATT = _mk_pack(
    [("dawA", 128, C4), ("dawB", 32, C4), ("pawA", 128, C4), ("pawB", 32, C4),
     ("awA", 128, C4), ("awB", 32, C4),
     ("dawrA", 128, 128), ("dawrB", 32, 128),
     ("pawrA", 128, 128), ("pawrB", 32, 128)])

PK_FC = _mk_pack(
    [("fc1_0", 128, 1024), ("fc1_1", 128, 1024),
     ("fc1_2", 128, 1024), ("fc1_3", 128, 1024)]
    + [(f"fc2_{g}", 128, 1024) for g in range(8)]
    + [(f"fc3_{g}", 128, 512) for g in range(8)]
    + [(f"outw_{g}", 128, 2) for g in range(4)])


def _build():
    from contextlib import ExitStack
    import concourse.bass as bass
    import concourse.tile as tile
    from concourse import bacc, mybir

    f32 = mybir.dt.float32
    bf16 = mybir.dt.bfloat16
    AF = mybir.ActivationFunctionType
    ALU = mybir.AluOpType
    AX = mybir.AxisListType

    nc = bacc.Bacc("TRN2", target_bir_lowering=False, debug=False)

    idx_d = nc.declare_dram_parameter("idx", [1, 1104], bf16, isOutput=False)
    pk_d = {}
    for pname, (layout, w), dt in [
        ("pk_f32", PK_F32, f32), ("pk_boot", PK_BOOT, bf16),
        ("pk_conv", PK_CONV, bf16), ("pk_att", PK_ATT, bf16),
        ("pk_fc", PK_FC, bf16),
    ]:
        pk_d[pname] = nc.declare_dram_parameter(pname, [128, w], dt, isOutput=False)
    out_d = nc.declare_dram_parameter("out", [2, 1], f32, isOutput=True)

    with tile.TileContext(nc) as tc, ExitStack() as ctx:
        wp = ctx.enter_context(tc.tile_pool(name="w", bufs=1))
        ap_ = ctx.enter_context(tc.tile_pool(name="a", bufs=1))
        tp = ctx.enter_context(tc.tile_pool(name="t", bufs=8))
        pp = ctx.enter_context(tc.tile_pool(name="p", bufs=4, space="PSUM"))
        pT = ctx.enter_context(tc.tile_pool(name="pT", bufs=1, space="PSUM"))

        # ---- coalesced loads: idx on the scalar HWDGE ring (runs in
        # parallel with the sync ring's packs), packs in order of use ----
        idx_t = ap_.tile([1, 1104], bf16, tag="idx")
        nc.scalar.dma_start(out=idx_t[:], in_=idx_d[:])
        pk_t = {}
        for pname, (layout, w), dt in [
            ("pk_boot", PK_BOOT, bf16), ("pk_f32", PK_F32, f32),
            ("pk_conv", PK_CONV, bf16), ("pk_att", PK_ATT, bf16),
            ("pk_fc", PK_FC, bf16),
        ]:
            t = wp.tile([128, w], dt, tag=pname)
            nc.sync.dma_start(out=t[:], in_=pk_d[pname][:])
            pk_t[pname] = t

        def sl(pname, name):
            layout, _ = {"pk_f32": PK_F32, "pk_boot": PK_BOOT,
                         "pk_conv": PK_CONV, "pk_att": PK_ATT,
                         "pk_fc": PK_FC}[pname]
            r, off, c = layout[name]
            return pk_t[pname][0:r, off:off + c]

        # No PE warm-up: the HW runs the PE at a 50% utilization duty cycle
        # for the first ~50us of accumulated PE activity regardless (throttle
        # telemetry shows util_limit=0.5), so warm-up matmuls only burn that
        # budget and delay the real work behind them in the PE queue.
        ones_t = sl("pk_boot", "ones")
        iota_t = sl("pk_f32", "iota")
        id_t = sl("pk_boot", "id128")
        id4_t = sl("pk_boot", "id4")

        # ---- one-hot + embedding ----
        def embed(idx_ap, nvocab, L, emb_ap, tag):
            e = ap_.tile([DIM, L], bf16, tag=f"e_{tag}")
            for l0 in range(0, L, 512):
                cs = min(512, L - l0)
                psb = pp.tile([nvocab, 512], f32, tag="ps")
                nc.tensor.matmul(psb[:, :cs], ones_t[:, :nvocab],
                                 idx_ap[:, l0:l0 + cs], start=True, stop=True)
                oh = tp.tile([nvocab, 512], bf16, tag="oh")
                nc.vector.tensor_scalar(out=oh[:, :cs], in0=psb[:, :cs],
                                        scalar1=iota_t[:nvocab, :], scalar2=None,
                                        op0=ALU.is_equal)
                pse = pp.tile([DIM, 512], f32, tag="ps")
                nc.tensor.matmul(pse[:, :cs], emb_ap, oh[:, :cs], start=True, stop=True)
                nc.scalar.copy(e[:, l0:l0 + cs], pse[:, :cs])
            return e

        de = embed(idx_t[:, 0:LD], 65, LD, sl("pk_boot", "embd"), "d")
        pe = embed(idx_t[:, LD:LD + LP], 26, LP, sl("pk_boot", "embp"), "p")

        # ---- conv stacks (bf16 in/out, f32 psum) ----
        def conv(x, Lout, K, wname, b_ap, cout, tag, oc=None):
            y = ap_.tile([cout, Lout], bf16, tag=tag)
            for l0 in range(0, Lout, 512):
                cs = min(512, Lout - l0)
                ps = pp.tile([cout, 512], f32, tag="ps")
                for k in range(K):
                    w = sl("pk_conv", f"{wname}_{k}")
                    if oc is not None:
                        w = w[:, oc[0]:oc[0] + oc[1]]
                    nc.tensor.matmul(ps[:, :cs], w, x[:, l0 + k:l0 + k + cs],
                                     start=(k == 0), stop=(k == K - 1))
                nc.scalar.activation(y[:, l0:l0 + cs], ps[:, :cs], AF.Relu, bias=b_ap)
            return y

        dc1 = conv(de, LD1, 4, "dw1", sl("pk_f32", "db1"), CONV, "dc1")
        dc2 = conv(dc1, LD2, 6, "dw2", sl("pk_f32", "db2"), 2 * CONV, "dc2")
        pc1 = conv(pe, LP1, 4, "pw1", sl("pk_f32", "pb1"), CONV, "pc1")
        pc2 = conv(pc1, LP2, 8, "pw2", sl("pk_f32", "pb2"), 2 * CONV, "pc2")

        # ---- fused conv3 + attention projections, chunked along L so the
        # projection matmuls/activations overlap the next conv3 chunk ----
        # out tiles: X_A [128, L] (chans 0:128) and X_B4 [128, L] (chans
        # 128:160 x4 lane-replicated). Protein att tiles are [128, LPP] with
        # col 979 = -1e4 so relu() of the pad contributes 0.
        def conv3_att(x, L, Lpad, K, wname, pfx, tag, dt_a):
            cc0 = ap_.tile([CH[0][1], L], bf16, tag=f"{tag}c0")
            cc1 = ap_.tile([CH[1][1], L], bf16, tag=f"{tag}c1")
            cc = [cc0, cc1]
            aA = ap_.tile([128, Lpad], dt_a, tag=f"{tag}a0")
            aB = ap_.tile([128, Lpad], dt_a, tag=f"{tag}a1")
            if Lpad > L:
                nc.vector.memset(aA[:, L:Lpad], -1e4)
                nc.vector.memset(aB[:, L:Lpad], -1e4)
            for l0 in range(0, L, 512):
                cs = min(512, L - l0)
                for j, s in ((0, "A"), (1, "B")):
                    o, w_ = CH[j]
                    ps = pp.tile([w_, 512], f32, tag="ps")
                    for k in range(K):
                        w = sl("pk_conv", f"{wname}_{k}")[:, o:o + w_]
                        nc.tensor.matmul(ps[:, :cs], w, x[:, l0 + k:l0 + k + cs],
                                         start=(k == 0), stop=(k == K - 1))
                    nc.scalar.activation(cc[j][:, l0:l0 + cs], ps[:, :cs], AF.Relu,
                                         bias=sl("pk_f32", f"{pfx}b3{s}"))
                for which, y in ((0, aA), (1, aB)):
                    ps = pp.tile([128, 512], f32, tag="ps")
                    for j, s in ((0, "A"), (1, "B")):
                        w = (sl("pk_att", f"{pfx}awA")[:, 0:128],
                             sl("pk_att", f"{pfx}awB")[:, 0:128])[j] if which == 0 \
                            else (sl("pk_att", f"{pfx}awrA"),
                                  sl("pk_att", f"{pfx}awrB"))[j]
                        nc.tensor.matmul(ps[:, :cs], w, cc[j][:, l0:l0 + cs],
                                         start=(j == 0), stop=(j == 1))
                    bias = sl("pk_f32", f"{pfx}abA") if which == 0 \
                        else sl("pk_f32", f"{pfx}abr")
                    nc.scalar.activation(y[:, l0:l0 + cs], ps[:, :cs], AF.Identity,
                                         bias=bias)
            return cc, aA, aB

        # D tiles f32 (used as per-partition scalars); P tiles bf16 (streamed)
        dc, D_A, D_B4 = conv3_att(dc2, LD3, LD3, 8, "dw3", "d", "dc3", f32)
        pc, P_A, P_B4 = conv3_att(pc2, LP3, LPP, 12, "pw3", "p", "pc3", bf16)

        # pack D_B4 [128, 85] -> D_Bp [128, 22]: lane (32g+c), col t = D[128+c, 4t+g]
        D_Bpad = ap_.tile([128, 88], f32, tag="D_Bpad")
        nc.vector.memset(D_Bpad[:], -1e4)
        nc.vector.tensor_copy(D_Bpad[:, 0:85], D_B4[:, 0:85])
        D_Bp = ap_.tile([128, NB], f32, tag="D_Bp")
        for g in range(4):
            nc.vector.tensor_copy(D_Bp[g * 32:(g + 1) * 32, :],
                                  D_Bpad[g * 32:(g + 1) * 32, g:88:4])

        # ---- R loops ----
        # tmp = relu(P + D[:, i]); S col via in-instruction accumulate;
        # T += tmp via identity matmul into PSUM. Producers alternate between
        # DVE scalar_tensor_tensor and ScalarE activation (both ~1.1-1.2us for
        # a [128,980] pass; no DVE fast mode exists for any op that can fuse
        # relu with a sum-accumulate).
        zeros_t = ap_.tile([128, LPP], bf16, tag="zeros")
        nc.vector.memset(zeros_t[:], 0.0)

        # Warm the Sigmoid activation table off the critical path (its
        # ACT_TABLE_LOAD is ~1.3us and would otherwise fire at first atte use)
        sig_wu = ap_.tile([1, 2], f32, tag="sig_wu")
        nc.scalar.activation(sig_wu[:], zeros_t[0:1, 0:2], AF.Sigmoid)

        # Global-max-pool vectors, pre-zeroed off the critical path. All are
        # [128, 1] (B-chunk rows 32:128 stay zero) so every fc1 matmul has the
        # same PE tile shape — mixed 128/32-row weight tiles forced a PE
        # reconfig between matmuls (~116ns vs ~27ns issue cadence).
        vecs = {}
        for vtag in ("d0", "d1", "p0", "p1"):
            v = ap_.tile([128, 1], bf16, tag=f"v_{vtag}")
            nc.vector.memset(v[:], 0.0)
            vecs[vtag] = v

        def r_loop(P_t, D_cols, n_iter, s_tile, psl, psh, id_tile, idw):
            for i in range(n_iter):
                tm = tp.tile([128, LPP], bf16, tag="rtmp")
                eng = R_SCHED[i % len(R_SCHED)]
                if eng == "A":
                    nc.scalar.activation(tm[:], P_t[:], AF.Relu,
                                         bias=D_cols[:, i:i + 1],
                                         accum_out=s_tile[:, i:i + 1])
                else:
                    nc.vector.scalar_tensor_tensor(
                        out=tm[:], in0=P_t[:], scalar=D_cols[:, i:i + 1],
                        in1=zeros_t[:], op0=ALU.add, op1=ALU.max,
                        accum_out=s_tile[:, i:i + 1])
                nc.tensor.matmul(psl[:], id_tile[:, :idw], tm[:, 0:512],
                                 start=(i == 0), stop=(i == n_iter - 1))
                nc.tensor.matmul(psh[:], id_tile[:, :idw], tm[:, 512:LPP],
                                 start=(i == 0), stop=(i == n_iter - 1))

        S_A = ap_.tile([128, LD3], f32, tag="S_A")
        TA0 = pT.tile([128, 512], f32, tag="TA0")
        TA1 = pT.tile([128, LPP - 512], f32, tag="TA1")
        r_loop(P_A, D_A, LD3, S_A, TA0, TA1, id_t, 128)

        S_B4 = ap_.tile([128, NB], f32, tag="S_B4")
        TB0 = pT.tile([32, 512], f32, tag="TB0")
        TB1 = pT.tile([32, LPP - 512], f32, tag="TB1")
        r_loop(P_B4, D_Bp, NB, S_B4, TB0, TB1, id4_t, 32)

        # S -> bf16 rhs tiles: S_Ab [128, 85]; unpack S_B4 -> S_Bb [32, 85]
        S_Ab = ap_.tile([128, LD3], bf16, tag="S_Ab")
        nc.vector.tensor_copy(S_Ab[:], S_A[:])
        S_Bb = ap_.tile([32, LD3], bf16, tag="S_Bb")
        for g in range(4):
            cnt = NB if g == 0 else NB - 1
            nc.vector.tensor_copy(S_Bb[:, g:g + 4 * (cnt - 1) + 1:4],
                                  S_B4[g * 32:(g + 1) * 32, 0:cnt])
        # T psum -> bf16 sbuf (pad col 979 dropped); A on ScalarE, B on DVE so
        # the two copies overlap
        T_Ab = ap_.tile([128, LP3], bf16, tag="T_Ab")
        nc.scalar.copy(T_Ab[:, 0:512], TA0[:])
        nc.scalar.copy(T_Ab[:, 512:LP3], TA1[:, 0:LP3 - 512])
        T_Bb = ap_.tile([32, LP3], bf16, tag="T_Bb")
        nc.vector.tensor_copy(T_Bb[:, 0:512], TB0[:])
        nc.vector.tensor_copy(T_Bb[:, 512:LP3], TB1[:, 0:LP3 - 512])
        S_ch = [S_Ab, S_Bb]
        T_ch = [T_Ab, T_Bb]

        # ---- attention outputs: sigmoid((sum/n) @ att_w + att_b) ----
        def atte(rhs_ch, L, scale, tag):
            res = []
            for which, (o, w) in enumerate(CH):
                y = ap_.tile([w, L], bf16, tag=f"{tag}{which}")
                for l0 in range(0, L, 512):
                    cs = min(512, L - l0)
                    ps = pp.tile([w, 512], f32, tag="ps")
                    for j, s in ((0, "A"), (1, "B")):
                        aw = sl("pk_att", f"aw{s}")
                        nc.tensor.matmul(ps[:, :cs], aw[:, o:o + w],
                                         rhs_ch[j][:, l0:l0 + cs],
                                         start=(j == 0), stop=(j == 1))
                    nc.scalar.activation(y[:, l0:l0 + cs], ps[:, :cs], AF.Sigmoid,
                                         bias=sl("pk_f32", f"ab{'AB'[which]}"),
                                         scale=scale)
                res.append(y)
            return res

        ca = atte(S_ch, LD3, 1.0 / LP3, "ca")
        pa = atte(T_ch, LP3, 1.0 / LD3, "pa")

        # ---- gate + global max pool: v = max_l(src * (0.5 + atte)) ----
        for (src, att_, L, tag) in [(dc, ca, LD3, "d"), (pc, pa, LP3, "p")]:
            for which, (o, w) in enumerate(CH):
                m = tp.tile([w, L], bf16, tag=f"m_{tag}{which}")
                nc.vector.scalar_tensor_tensor(
                    out=m[:], in0=att_[which][:], scalar=0.5,
                    in1=src[which][:, 0:L], op0=ALU.add, op1=ALU.mult)
                nc.vector.reduce_max(vecs[f"{tag}{which}"][0:w, :], m[:], axis=AX.X)
        # pair layout: [dvecA(128), dvecB(pad), pvecA(128), pvecB(pad)]
        vlist = [vecs["d0"], vecs["d1"], vecs["p0"], vecs["p1"]]

        # ---- FC head ----
        def lrelu_bias(ps, b_ap, ncols, tag):
            h = ap_.tile([128, ncols], f32, tag=f"h_{tag}")
            nc.vector.tensor_tensor(out=h[:], in0=ps[:, :ncols], in1=b_ap, op=ALU.add)
            t1 = tp.tile([128, ncols], f32, tag="fct")
            nc.vector.tensor_scalar(out=t1[:], in0=h[:], scalar1=0.01, scalar2=None,
                                    op0=ALU.mult)
            h2 = ap_.tile([128, ncols], bf16, tag=f"h2_{tag}")
            nc.vector.tensor_tensor(out=h2[:], in0=h[:], in1=t1[:], op=ALU.max)
            return h2

        ps1 = pp.tile([128, 8], f32, tag="ps")
        for oc in range(8):
            for g in range(4):
                w = sl("pk_fc", f"fc1_{g}")
                nc.tensor.matmul(ps1[:, oc:oc + 1], w[:, oc * 128:(oc + 1) * 128],
                                 vlist[g][:], start=(g == 0), stop=(g == 3))
        h1 = lrelu_bias(ps1, sl("pk_f32", "fc1b"), 8, "1")

        ps2 = pp.tile([128, 8], f32, tag="ps")
        for oc in range(8):
            for g in range(8):
                w = sl("pk_fc", f"fc2_{g}")
                nc.tensor.matmul(ps2[:, oc:oc + 1], w[:, oc * 128:(oc + 1) * 128],
                                 h1[:, g:g + 1], start=(g == 0), stop=(g == 7))
        h2 = lrelu_bias(ps2, sl("pk_f32", "fc2b"), 8, "2")

        ps3 = pp.tile([128, 4], f32, tag="ps")
        for oc in range(4):
            for g in range(8):
                w = sl("pk_fc", f"fc3_{g}")
                nc.tensor.matmul(ps3[:, oc:oc + 1], w[:, oc * 128:(oc + 1) * 128],
                                 h2[:, g:g + 1], start=(g == 0), stop=(g == 7))
        h3 = lrelu_bias(ps3, sl("pk_f32", "fc3b"), 4, "3")

        pso = pp.tile([2, 1], f32, tag="ps")
        for g in range(4):
            nc.tensor.matmul(pso[:], sl("pk_fc", f"outw_{g}"), h3[:, g:g + 1],
                             start=(g == 0), stop=(g == 3))
        ob = ap_.tile([2, 1], f32, tag="ob")
        nc.scalar.activation(ob[:], pso[:], AF.Identity, bias=sl("pk_f32", "outb"))
        nc.sync.dma_start(out=out_d[:], in_=ob[:])

    nc.compile()
    return nc


def _prep_inputs(inputs):
    """Host-side layout prep. Returns (shared_params, per_core_fn)."""
    import ml_dtypes
    bf = ml_dtypes.bfloat16
    asn = np.asarray
    rep4 = lambda x: np.tile(x, (4,) + (1,) * (x.ndim - 1))

    vals = {}
    # f32 pack values
    vals["iota"] = np.arange(128, dtype=np.float32).reshape(128, 1)
    for nm, src in [("db1", "db1"), ("db2", "db2"), ("pb1", "pb1"), ("pb2", "pb2")]:
        vals[nm] = asn(inputs[src], dtype=np.float32).reshape(-1, 1)
    for nm, src in [("db3", "db3"), ("pb3", "pb3"), ("dab", "d_att_b"),
                    ("pab", "p_att_b"), ("ab", "att_b")]:
        v = asn(inputs[src], dtype=np.float32).reshape(-1, 1)
        vals[nm + "A"], vals[nm + "B"] = v[0:128], v[128:160]
    vals["dabr"] = rep4(asn(inputs["d_att_b"], dtype=np.float32)[128:160]).reshape(128, 1)
    vals["pabr"] = rep4(asn(inputs["p_att_b"], dtype=np.float32)[128:160]).reshape(128, 1)
    vals["fc1b"] = asn(inputs["fc1_b"], dtype=np.float32).reshape(8, 128).T.copy()
    vals["fc2b"] = asn(inputs["fc2_b"], dtype=np.float32).reshape(8, 128).T.copy()
    vals["fc3b"] = asn(inputs["fc3_b"], dtype=np.float32).reshape(4, 128).T.copy()
    vals["outb"] = asn(inputs["out_b"], dtype=np.float32).reshape(2, 1)
    # boot pack
    vals["ones"] = np.ones((1, 128), np.float32)
    vals["embd"] = asn(inputs["drug_emb"])
    vals["embp"] = asn(inputs["prot_emb"])
    vals["id128"] = np.eye(128, dtype=np.float32)
    vals["id4"] = np.tile(np.eye(32, dtype=np.float32), (4, 1))
    # conv pack: tap k of w [Cout, Cin, K] -> [Cin, Cout]
    for nm, src, K in [("dw1", "dw1", 4), ("dw2", "dw2", 6), ("dw3", "dw3", 8),
                       ("pw1", "pw1", 4), ("pw2", "pw2", 8), ("pw3", "pw3", 12)]:
        w = asn(inputs[src])
        for k in range(K):
            vals[f"{nm}_{k}"] = w[:, :, k].T
    # att pack
    for nm, src in [("daw", "d_att_w"), ("paw", "p_att_w"), ("aw", "att_w")]:
        w = asn(inputs[src])
        vals[nm + "A"], vals[nm + "B"] = w[0:128], w[128:160]
    for nm, src in [("dawr", "d_att_w"), ("pawr", "p_att_w")]:
        w = np.tile(asn(inputs[src])[:, 128:160], (1, 4))
        vals[nm + "A"], vals[nm + "B"] = w[0:128], w[128:160]
    # fc pack
    fc1 = asn(inputs["fc1_w"])
    vals["fc1_0"], vals["fc1_1"] = fc1[0:128], fc1[128:160]
    vals["fc1_2"], vals["fc1_3"] = fc1[160:288], fc1[288:320]
    fc2, fc3 = asn(inputs["fc2_w"]), asn(inputs["fc3_w"])
    for g in range(8):
        vals[f"fc2_{g}"] = fc2[g * 128:(g + 1) * 128]
        vals[f"fc3_{g}"] = fc3[g * 128:(g + 1) * 128]
    outw = asn(inputs["out_w"])
    for g in range(4):
        vals[f"outw_{g}"] = outw[g * 128:(g + 1) * 128]

    shared = {}
    for pname, (layout, w), dt in [
        ("pk_f32", PK_F32, np.float32), ("pk_boot", PK_BOOT, bf),
        ("pk_conv", PK_CONV, bf), ("pk_att", PK_ATT, bf), ("pk_fc", PK_FC, bf),
    ]:
        buf = np.zeros((128, w), dtype=dt)
        for name, (r, off, c) in layout.items():
            v = vals[name]
            buf[0:v.shape[0], off:off + c] = v
        shared[pname] = buf

    drug = asn(inputs["drug"]).astype(bf)
    prot = asn(inputs["protein"]).astype(bf)

    def per_core(i):
        m = dict(shared)
        idx = np.zeros((1, 1104), dtype=bf)
        idx[0, 0:LD] = drug[i]
        idx[0, LD:LD + LP] = prot[i]
        m["idx"] = idx
        return m

    return shared, per_core


def kernel(**inputs):
    import os
    # A NeuronCore left in a degraded DVFS state by a previous crash runs
    # ~20% slower; request a core reset on runtime init (no-op if the
    # harness already set a policy).
    os.environ.setdefault("NEURON_RT_RESET_CORES", "1")
    from concourse.bass_utils import run_bass_kernel_spmd

    if "nc" not in _CACHE:
        _CACHE["nc"] = _build()
    nc = _CACHE["nc"]
    _, per_core = _prep_inputs(inputs)
    in_maps = [per_core(i) for i in range(B)]
    r = run_bass_kernel_spmd(nc, in_maps, core_ids=list(range(B)))
    out = np.stack([r.results[i]["out"].reshape(2) for i in range(B)])
    return out.astype(np.float32)
